# revision 1
# baseline (speedup 1.0000x reference)
"""Trainium2 Bass kernel for nn_Cross_Attention (8-core data-parallel over batch).

Reference computation per batch item:
  kvf  = conv1x1(kv, qkv1_w)                    # [384, H, W]
  kvd  = depthwise3x3(kvf, qkv2_w, pad=1)       # [384, H, W]
  k, v = split(kvd); qh/kh L2-normalized over hw per head-channel row
  attn = softmax(scale * qn @ kn^T)             # [8, 24, 24] block per head
  out  = proj1x1(attn @ v, proj_w)              # [192, H, W]

Each NeuronCore processes one batch item end-to-end (no collectives).
SBUF pressure is managed with one bufs=1 pool whose tags alias big tensors
with disjoint lifetimes (q16 reuses k16's slots, kB reuses kvf's slot).
"""

import os
import sys

sys.path.insert(0, "/opt/trn_rl_repo")

import numpy as np

import concourse.bass as bass
import concourse.tile as tile
from concourse import bacc, mybir
from concourse.bass_utils import run_bass_kernel_spmd
from concourse.bass_interp import get_hw_module

F32 = mybir.dt.float32
F16 = mybir.dt.float16

# Problem dims (per core / batch item)
C = 192          # input channels
C2 = 384         # conv1 output channels
HEADS = 8
CD = C // HEADS  # 24 channels per head
W = 128          # image cols (== partition width for pixel tiles)
H = int(os.environ.get("BASS_CA_H", "128"))  # image rows (overridable for sim)
HWTOT = H * W
PT = 512                    # pixels per matmul tile (one PSUM bank fp32)
RPT = PT // W               # image rows per tile (4)
NT = HWTOT // PT            # pixel tiles (32 at H=128)
PADR = 2                    # zero-pad rows on each side of kvf (covers dr=+-1)
EPS = 1e-12

# taps: (dr, dc), weight index = (dr+1)*3 + (dc+1); (0,0) first so the
# first matmul of each PSUM accumulation group writes every element.
TAPS = [(0, 0)] + [(dr, dc) for dr in (-1, 0, 1) for dc in (-1, 0, 1)
                   if not (dr == 0 and dc == 0)]


def sl(nt, size=PT):
    return slice(nt * size, (nt + 1) * size)


def emit_kernel(tc, io):
    nc = tc.nc
    kv, q, w1t, w2d, wpt, ident, mask, scale192 = (
        io["kv"], io["q"], io["w1t"], io["w2d"], io["wpt"], io["ident"],
        io["mask"], io["scale192"])
    out = io["out"]
    vdram = io["vdram"]
    kv16d = io["kv16d"]

    from contextlib import ExitStack
    _stack = ExitStack()
    wp = _stack.enter_context(tc.tile_pool(name="weights", bufs=1))
    sml = _stack.enter_context(tc.tile_pool(name="small", bufs=1))
    big = _stack.enter_context(tc.tile_pool(name="big", bufs=1))

    # ---- weights to SBUF ----
    w1ta = wp.tile([128, C2], F16); nc.sync.dma_start(w1ta[:], w1t[0:128, :])
    w1tb = wp.tile([64, C2], F16); nc.sync.dma_start(w1tb[:], w1t[128:C, :])
    wpta = wp.tile([128, C], F16); nc.sync.dma_start(wpta[:], wpt[0:128, :])
    wptb = wp.tile([64, C], F16); nc.sync.dma_start(wptb[:], wpt[128:C, :])
    id16 = wp.tile([128, 128], F16); nc.sync.dma_start(id16[:], ident[:])
    maska = wp.tile([128, C], F32); nc.sync.dma_start(maska[:], mask[0:128, :])
    maskb = wp.tile([64, C], F32); nc.sync.dma_start(maskb[:], mask[128:C, :])
    sca = wp.tile([128, 1], F32); nc.sync.dma_start(sca[:], scale192[0:128, :])
    scb = wp.tile([64, 1], F32); nc.sync.dma_start(scb[:], scale192[128:C, :])
    w2sb = wp.tile([128, 27, 128], F16)
    nc.sync.dma_start(w2sb[:], w2d.rearrange("t p c -> p t c"))

    spa = sml.tile([128, 1], F32)
    spb = sml.tile([64, 1], F32)
    bda = sml.tile([128, C], F16)
    bdb = sml.tile([64, C], F16)

    # ---- stage 0: kv f32 -> f16 copy in DRAM (conv1 re-reads it 3x) ----
    with tc.tile_pool(name="cvt", bufs=3) as cst:
        for nt in range(NT):
            sa = cst.tile([128, PT], F32, tag="sa")
            nc.sync.dma_start(sa[:], kv[0:128, sl(nt)])
            fa = cst.tile([128, PT], F16, tag="fa")
            nc.any.tensor_copy(fa[:], sa[:])
            nc.sync.dma_start(kv16d[0:128, sl(nt)], fa[:])
            sb = cst.tile([64, PT], F32, tag="sb")
            nc.sync.dma_start(sb[:], kv[128:C, sl(nt)])
            fb = cst.tile([64, PT], F16, tag="fb")
            nc.any.tensor_copy(fb[:], sb[:])
            nc.sync.dma_start(kv16d[128:C, sl(nt)], fb[:])

    # ================= conv1 + depthwise =================
    k16a = big.tile([128, HWTOT], F16, tag="slot_ka")
    k16b = big.tile([64, HWTOT], F16, tag="slot_kb")

    with tc.tile_pool(name="convst", bufs=3) as st, \
         tc.tile_pool(name="psA", bufs=2, space="PSUM") as psA, \
         tc.tile_pool(name="psB", bufs=1, space="PSUM") as psB, \
         tc.tile_pool(name="vstage", bufs=2) as vst:
        for mc in range(3):
            kvf = big.tile([128, (H + 2 * PADR) * W], F16, tag="slot_kvf",
                           name="kvf")
            kvf3 = kvf[:].rearrange("p (r c) -> p r c", c=W)
            nc.vector.memset(kvf3[:, 0:PADR, :], 0.0)
            nc.vector.memset(kvf3[:, PADR + H:, :], 0.0)
            # conv1: kvf[mc*128 + ch, pix] = sum_cin w1[ch, cin] kv[cin, pix]
            for nt in range(NT):
                ka = st.tile([128, PT], F16, tag="ka")
                nc.sync.dma_start(ka[:], kv16d[0:128, sl(nt)])
                kb = st.tile([64, PT], F16, tag="kb")
                nc.sync.dma_start(kb[:], kv16d[128:C, sl(nt)])
                ps = psA.tile([128, PT], F32, tag="psA")
                nc.tensor.matmul(ps[:], w1ta[:, mc * 128:(mc + 1) * 128],
                                 ka[:], start=True, stop=False)
                nc.tensor.matmul(ps[:], w1tb[:, mc * 128:(mc + 1) * 128],
                                 kb[:], start=False, stop=True)
                nc.any.tensor_copy(kvf[:, PADR * W + nt * PT:
                                       PADR * W + (nt + 1) * PT], ps[:])
            # depthwise 3x3 via diagonal-weight matmuls, accumulate in PSUM.
            # PSUM pixel tiles are col-major ([c*RPT + r]) so dc-shifted
            # output slices stay flat-contiguous (1 free dim).
            kvfT3 = kvf[:].rearrange("p (r c) -> p c r", c=W)
            for g in range(0, NT, 6):
                gn = min(6, NT - g)
                pss = [psB.tile([128, PT], F32, tag=f"psB{j}",
                                name=f"psB{j}") for j in range(gn)]
                for ti, (dr, dc) in enumerate(TAPS):
                    wi = (dr + 1) * 3 + (dc + 1)
                    lw = w2sb[:, mc * 9 + wi, :]
                    if dc == 0:
                        ci, co = slice(0, W), slice(0, PT)
                    elif dc == -1:
                        ci, co = slice(0, W - 1), slice(RPT, PT)
                    else:
                        ci, co = slice(1, W), slice(0, PT - RPT)
                    for j in range(gn):
                        r0 = (g + j) * RPT
                        rs = slice(PADR + r0 + dr, PADR + r0 + dr + RPT)
                        nc.tensor.matmul(pss[j][:, co], lw, kvfT3[:, ci, rs],
                                         start=(ti == 0), stop=(ti == 8))
                # evacuate to k (ch < 192) and v (ch >= 192); destination
                # views are col-major to match the PSUM layout
                for j in range(gn):
                    nt = g + j
                    pcm = pss[j][:]

                    def cmv(ap):
                        return ap.rearrange("p (r c) -> p c r", r=RPT)
                    if mc == 0:
                        nc.any.tensor_copy(cmv(k16a[:, sl(nt)]), pcm)
                    elif mc == 1:
                        nc.any.tensor_copy(cmv(k16b[:, sl(nt)]), pcm[0:64, :])
                        vs = vst.tile([128, PT], F16, tag="vs")
                        nc.any.tensor_copy(cmv(vs[64:128, :]), pcm[64:128, :])
                        nc.sync.dma_start(vdram[0:64, sl(nt)], vs[64:128, :])
                    else:
                        vs = vst.tile([128, PT], F16, tag="vs")
                        nc.any.tensor_copy(cmv(vs[:]), pcm)
                        nc.sync.dma_start(vdram[64:C, sl(nt)], vs[:])

    # ================= k norms, scale, transpose =================
    with tc.tile_pool(name="norm", bufs=1) as npl:
        nk2a = npl.tile([128, 1], F32)
        nk2b = npl.tile([64, 1], F32)
        NCH = 8
        CHW = HWTOT // NCH
        kparts_a = npl.tile([128, NCH], F32)
        kparts_b = npl.tile([64, NCH], F32)
        sqs = npl.tile([128, CHW], F16)
        for i in range(NCH):
            nc.scalar.activation(sqs[:, :], k16a[:, sl(i, CHW)],
                                 mybir.ActivationFunctionType.Square,
                                 accum_out=kparts_a[:, i:i + 1])
        for i in range(NCH):
            nc.scalar.activation(sqs[0:64, :], k16b[:, sl(i, CHW)],
                                 mybir.ActivationFunctionType.Square,
                                 accum_out=kparts_b[:, i:i + 1])
        nc.vector.reduce_sum(nk2a[:], kparts_a[:], axis=mybir.AxisListType.X)
        nc.vector.reduce_sum(nk2b[:], kparts_b[:], axis=mybir.AxisListType.X)
        for nk2 in (nk2a, nk2b):
            nc.scalar.sqrt(nk2[:], nk2[:])
            nc.vector.tensor_scalar_max(nk2[:], nk2[:], EPS)
            nc.vector.reciprocal(nk2[:], nk2[:])
        nc.vector.tensor_scalar_mul(k16a[:], k16a[:], nk2a[:])
        nc.vector.tensor_scalar_mul(k16b[:], k16b[:], nk2b[:])

        # kBa reuses kvf's slot (kvf is dead after the depthwise)
        kBa = big.tile([128, H, 128], F16, tag="slot_kvf", name="kBa")
        kBb = big.tile([128, H, 64], F16, tag="slot_kbb", name="kBb")
        nc.sync.dma_start_transpose(kBa[:], k16a[:])
        nc.sync.dma_start_transpose(kBb[:], k16b[:])

        # ========== q: load, norms (q16 reuses k16 slots) ==========
        q16a = big.tile([128, HWTOT], F16, tag="slot_ka", name="q16a")
        q16b = big.tile([64, HWTOT], F16, tag="slot_kb", name="q16b")
        qpa = npl.tile([128, NT], F32)
        qpb = npl.tile([64, NT], F32)
        with tc.tile_pool(name="qstage", bufs=3) as qst:
            for nt in range(NT):
                sa = qst.tile([128, PT], F32, tag="qsa")
                nc.sync.dma_start(sa[:], q[0:128, sl(nt)])
                nc.any.tensor_copy(q16a[:, sl(nt)], sa[:])
                qsq = qst.tile([128, PT], F16, tag="qsq")
                nc.scalar.activation(qsq[:], q16a[:, sl(nt)],
                                     mybir.ActivationFunctionType.Square,
                                     accum_out=qpa[:, nt:nt + 1])
                sb = qst.tile([64, PT], F32, tag="qsb")
                nc.sync.dma_start(sb[:], q[128:C, sl(nt)])
                nc.any.tensor_copy(q16b[:, sl(nt)], sb[:])
                nc.scalar.activation(qsq[0:64, :], q16b[:, sl(nt)],
                                     mybir.ActivationFunctionType.Square,
                                     accum_out=qpb[:, nt:nt + 1])
        nq2a = npl.tile([128, 1], F32)
        nq2b = npl.tile([64, 1], F32)
        nc.vector.reduce_sum(nq2a[:], qpa[:], axis=mybir.AxisListType.X)
        nc.vector.reduce_sum(nq2b[:], qpb[:], axis=mybir.AxisListType.X)
        for nq2, spx, scx in ((nq2a, spa, sca), (nq2b, spb, scb)):
            nc.scalar.sqrt(nq2[:], nq2[:])
            nc.vector.tensor_scalar_max(nq2[:], nq2[:], EPS)
            nc.vector.reciprocal(nq2[:], nq2[:])
            nc.vector.tensor_tensor(out=spx[:], in0=nq2[:], in1=scx[:],
                                    op=mybir.AluOpType.mult)

        qBa = big.tile([128, H, 128], F16, tag="slot_qba", name="qBa")
        qBb = big.tile([128, H, 64], F16, tag="slot_qbb", name="qBb")
        nc.sync.dma_start_transpose(qBa[:], q16a[:])
        nc.sync.dma_start_transpose(qBb[:], q16b[:])

    # ================= Gram =================
    with tc.tile_pool(name="psG", bufs=1, space="PSUM") as psG, \
         tc.tile_pool(name="smax", bufs=1) as sm, \
         tc.tile_pool(name="psT", bufs=1, space="PSUM") as psT:
        G0a = psG.tile([128, 128], F32, tag="G0a", name="G0a")
        G0b = psG.tile([128, 64], F32, tag="G0b", name="G0b")
        G1a = psG.tile([64, 128], F32, tag="G1a", name="G1a")
        G1b = psG.tile([64, 64], F32, tag="G1b", name="G1b")
        for t in range(H):
            s0, s1 = (t == 0), (t == H - 1)
            nc.tensor.matmul(G0a[:], qBa[:, t, :], kBa[:, t, :],
                             start=s0, stop=s1)
            nc.tensor.matmul(G0b[:], qBa[:, t, :], kBb[:, t, :],
                             start=s0, stop=s1)
            nc.tensor.matmul(G1a[:], qBb[:, t, :], kBa[:, t, :],
                             start=s0, stop=s1)
            nc.tensor.matmul(G1b[:], qBb[:, t, :], kBb[:, t, :],
                             start=s0, stop=s1)

        # ---- masked softmax over d (free dim), build block-diag attn^T ----
        for Ga, Gb, spx, mkx, rows in ((G0a, G0b, spa, maska, 128),
                                       (G1a, G1b, spb, maskb, 64)):
            lg = sm.tile([rows, C], F32, tag=f"lg{rows}", name=f"lg{rows}")
            nc.vector.scalar_tensor_tensor(
                out=lg[:, 0:128], in0=Ga[:], scalar=spx[:], in1=mkx[:, 0:128],
                op0=mybir.AluOpType.mult, op1=mybir.AluOpType.add)
            nc.vector.scalar_tensor_tensor(
                out=lg[:, 128:C], in0=Gb[:], scalar=spx[:], in1=mkx[:, 128:C],
                op0=mybir.AluOpType.mult, op1=mybir.AluOpType.add)
            mx = sm.tile([rows, 1], F32, tag=f"mx{rows}", name=f"mx{rows}")
            nc.vector.reduce_max(mx[:], lg[:], axis=mybir.AxisListType.X)
            nc.vector.tensor_scalar_mul(mx[:], mx[:], -1.0)
            ssum = sm.tile([rows, 1], F32, tag=f"ss{rows}", name=f"ss{rows}")
            nc.scalar.activation(lg[:], lg[:], mybir.ActivationFunctionType.Exp,
                                 bias=mx[:], accum_out=ssum[:])
            nc.vector.reciprocal(ssum[:], ssum[:])
            at16 = sm.tile([rows, C], F16, tag=f"at{rows}", name=f"at{rows}")
            nc.vector.tensor_scalar_mul(at16[:], lg[:], ssum[:])
            # transpose [rows, C] attn block into BD tiles
            tp0 = psT.tile([128, 128], F16, tag="tp0", name="tp0")
            nc.tensor.transpose(tp0[0:128, 0:rows], at16[:, 0:128],
                                id16[0:rows, 0:rows])
            tp1 = psT.tile([128, 128], F16, tag="tp1", name="tp1")
            nc.tensor.transpose(tp1[0:64, 0:rows], at16[:, 128:C],
                                id16[0:rows, 0:rows])
            if rows == 128:
                nc.any.tensor_copy(bda[:, 0:128], tp0[0:128, 0:128])
                nc.any.tensor_copy(bdb[:, 0:128], tp1[0:64, 0:128])
            else:
                nc.any.tensor_copy(bda[:, 128:C], tp0[0:128, 0:64])
                nc.any.tensor_copy(bdb[:, 128:C], tp1[0:64, 0:64])

    # ================= O = attn @ v, then proj =================
    with tc.tile_pool(name="ostage", bufs=3) as ost, \
         tc.tile_pool(name="psO", bufs=1, space="PSUM") as psO:
        for nt in range(NT):
            va = ost.tile([128, PT], F16, tag="va")
            nc.sync.dma_start(va[:], vdram[0:128, sl(nt)])
            vb = ost.tile([64, PT], F16, tag="vb")
            nc.sync.dma_start(vb[:], vdram[128:C, sl(nt)])
            O0 = psO.tile([128, PT], F32, tag="O0")
            O1 = psO.tile([64, PT], F32, tag="O1")
            nc.tensor.matmul(O0[:], bda[:, 0:128], va[:], start=True, stop=False)
            nc.tensor.matmul(O0[:], bdb[:, 0:128], vb[:], start=False, stop=True)
            nc.tensor.matmul(O1[:], bda[:, 128:C], va[:], start=True, stop=False)
            nc.tensor.matmul(O1[:], bdb[:, 128:C], vb[:], start=False, stop=True)
            oa = ost.tile([128, PT], F16, tag="oa")
            ob = ost.tile([64, PT], F16, tag="ob")
            nc.any.tensor_copy(oa[:], O0[:])
            nc.any.tensor_copy(ob[:], O1[:])
            P0 = psO.tile([128, PT], F32, tag="P0")
            P1 = psO.tile([64, PT], F32, tag="P1")
            nc.tensor.matmul(P0[:], wpta[:, 0:128], oa[:], start=True, stop=False)
            nc.tensor.matmul(P0[:], wptb[:, 0:128], ob[:], start=False, stop=True)
            nc.tensor.matmul(P1[:], wpta[:, 128:C], oa[:], start=True, stop=False)
            nc.tensor.matmul(P1[:], wptb[:, 128:C], ob[:], start=False, stop=True)
            fa = ost.tile([128, PT], F32, tag="fa")
            fb = ost.tile([64, PT], F32, tag="fb")
            nc.any.tensor_copy(fa[:], P0[:])
            nc.any.tensor_copy(fb[:], P1[:])
            nc.sync.dma_start(out[0:128, sl(nt)], fa[:])
            nc.sync.dma_start(out[128:C, sl(nt)], fb[:])
    _stack.close()


def build_module():
    nc = bacc.Bacc("TRN2")
    io = {}
    io["kv"] = nc.dram_tensor("kv", [C, HWTOT], F32, kind="ExternalInput").ap()
    io["q"] = nc.dram_tensor("q", [C, HWTOT], F32, kind="ExternalInput").ap()
    io["w1t"] = nc.dram_tensor("w1t", [C, C2], F16, kind="ExternalInput").ap()
    io["w2d"] = nc.dram_tensor("w2d", [27, 128, 128], F16, kind="ExternalInput").ap()
    io["wpt"] = nc.dram_tensor("wpt", [C, C], F16, kind="ExternalInput").ap()
    io["ident"] = nc.dram_tensor("ident", [128, 128], F16, kind="ExternalInput").ap()
    io["mask"] = nc.dram_tensor("mask", [C, C], F32, kind="ExternalInput").ap()
    io["scale192"] = nc.dram_tensor("scale192", [C, 1], F32, kind="ExternalInput").ap()
    io["out"] = nc.dram_tensor("out", [C, HWTOT], F32, kind="ExternalOutput").ap()
    io["vdram"] = nc.dram_tensor("vdram", [C, HWTOT], F16).ap()
    io["kv16d"] = nc.dram_tensor("kv16d", [C, HWTOT], F16).ap()
    with tile.TileContext(nc) as tc:
        emit_kernel(tc, io)
    nc.compile()
    return nc


def prep_weights(qkv1_w, qkv2_w, proj_w, scale):
    w1 = np.asarray(qkv1_w).reshape(C2, C)
    w1t = np.ascontiguousarray(w1.T).astype(np.float16)
    w2 = np.asarray(qkv2_w).reshape(C2, 9)
    w2d = np.zeros((27, 128, 128), np.float16)
    for mc in range(3):
        for wi in range(9):
            np.fill_diagonal(w2d[mc * 9 + wi], w2[mc * 128:(mc + 1) * 128, wi])
    wp = np.asarray(proj_w).reshape(C, C)
    wpt = np.ascontiguousarray(wp.T).astype(np.float16)
    ident = np.eye(128, dtype=np.float16)
    mask = np.full((C, C), -1e30, np.float32)
    for h in range(HEADS):
        mask[h * CD:(h + 1) * CD, h * CD:(h + 1) * CD] = 0.0
    scale192 = np.repeat(np.asarray(scale).reshape(HEADS), CD).astype(
        np.float32).reshape(C, 1)
    return {"w1t": w1t, "w2d": w2d, "wpt": wpt, "ident": ident,
            "mask": mask, "scale192": scale192}


_CACHED = {}


def kernel(kv, q, qkv1_w, qkv2_w, proj_w, scale):
    kv = np.asarray(kv, np.float32)
    q = np.asarray(q, np.float32)
    b = kv.shape[0]
    assert b == 8 and kv.shape[1] == C
    wts = prep_weights(qkv1_w, qkv2_w, proj_w, scale)
    if "nc" not in _CACHED:
        nc = build_module()
        nc.m = get_hw_module(nc.m)
        _CACHED["nc"] = nc
    nc = _CACHED["nc"]
    in_maps = []
    for i in range(b):
        m = {"kv": np.ascontiguousarray(kv[i].reshape(C, HWTOT)),
             "q": np.ascontiguousarray(q[i].reshape(C, HWTOT))}
        m.update(wts)
        in_maps.append(m)
    res = run_bass_kernel_spmd(nc, in_maps, core_ids=list(range(8)))
    out = np.stack([res.results[i]["out"].reshape(C, H, W) for i in range(b)])
    return out.astype(np.float32)



# revision 13
# speedup vs baseline: 2.2739x; 2.2739x over previous
"""Trainium2 Bass kernel for nn_Cross_Attention (8-core data-parallel over batch).

Per batch item (one NeuronCore):
  kvf  = conv1x1(kv, qkv1_w)                    # [384, H, W]
  kvd  = depthwise3x3(kvf, qkv2_w, pad=1)       # [384, H, W]
  k, v = split(kvd)
  G    = q_raw @ k_raw^T  (full 192x192 Gram, contracted over pixels)
  attn = softmax(G * scale/|q_i| * 1/|k_j| + blockdiag_mask)
  out  = (Wp @ attn) @ v          # proj folded into attention matrix

Key structure:
 - depthwise as diagonal-weight matmuls with a row-padded kvf layout
   ([130 rows x 130 cols] per 128-ch block) so every tap's rhs is a
   [4,128]-stride-130 AP (256B runs -> full PE stream rate).
 - mc block order (1, 2, 0): k channels 128-191 finish early, k 0-127
   are produced last and Gram accumulation (on raw q/k) is interleaved
   chunk-by-chunk with the last depthwise block. Norms are folded into
   the logits afterwards (row scale = per-partition scalar; column
   scale 1/|k_j| via a K=1 outer-product matmul broadcast).
 - v spilled to DRAM as f16, read back during attn@v.
 - f32<->f16 casts ride inside SWDGE DMAs (gpsimd).
"""

import sys

sys.path.insert(0, "/opt/trn_rl_repo")

import numpy as np

import concourse.bass as bass
import concourse.tile as tile
from concourse import bacc, mybir
from concourse.bass_utils import run_bass_kernel_spmd
from concourse.bass_interp import get_hw_module

F32 = mybir.dt.float32
F16 = mybir.dt.float16

C = 192          # input channels
C2 = 384         # conv1 output channels
HEADS = 8
CD = C // HEADS
W = 128
H = 128
HWTOT = H * W    # 16384
PT = 512         # pixels per matmul tile
NT = HWTOT // PT  # 32
RPT = PT // W    # 4 rows per tile
LC = 130         # padded kvf cols (1 left pad, 1 right pad)
LR = 130         # padded kvf rows
EPS = 1e-12
MC_ORDER = (1, 2, 0)
NCH = 8          # norm/Gram chunks
CHW = HWTOT // NCH  # 2048 pixels per chunk
TCH = CHW // W   # 16 t-steps per chunk

TAPS = [(dr, dc) for dr in (-1, 0, 1) for dc in (-1, 0, 1)]


def sl(i, size=PT):
    return slice(i * size, (i + 1) * size)


def emit_kernel(tc, io):
    nc = tc.nc
    kv, q = io["kv"], io["q"]
    w1t, w2d, wpt, mask = io["w1t"], io["w2d"], io["wpt"], io["mask"]
    scale192, ident = io["scale192"], io["ident"]
    out, vdram = io["out"], io["vdram"]

    from contextlib import ExitStack
    _stack = ExitStack()
    wp = _stack.enter_context(tc.tile_pool(name="weights", bufs=1))
    big = _stack.enter_context(tc.tile_pool(name="big", bufs=1))
    sml = _stack.enter_context(tc.tile_pool(name="small", bufs=1))

    # ---- weights ----
    w1ta = wp.tile([128, C2], F16); nc.sync.dma_start(w1ta[:], w1t[0:128, :])
    w1tb = wp.tile([64, C2], F16); nc.sync.dma_start(w1tb[:], w1t[128:C, :])
    wpta = wp.tile([128, C], F16); nc.sync.dma_start(wpta[:], wpt[0:128, :])
    wptb = wp.tile([64, C], F16); nc.sync.dma_start(wptb[:], wpt[128:C, :])
    maska = wp.tile([128, C], F32); nc.sync.dma_start(maska[:], mask[0:128, :])
    maskb = wp.tile([64, C], F32); nc.sync.dma_start(maskb[:], mask[128:C, :])
    sca = wp.tile([128, 1], F32); nc.sync.dma_start(sca[:], scale192[0:128, :])
    scb = wp.tile([64, 1], F32); nc.sync.dma_start(scb[:], scale192[128:C, :])
    id16 = wp.tile([128, 128], F16); nc.sync.dma_start(id16[:], ident[:])
    w2sb = wp.tile([128, 27, 128], F16)
    nc.sync.dma_start(w2sb[:], w2d.rearrange("t p c -> p t c"))
    ones1 = wp.tile([1, 128], F16); nc.vector.memset(ones1[:], 1.0)

    # ---- big persistent tiles ----
    kv16 = big.tile([128, 2 * HWTOT], F16, tag="slotA", name="kv16")
    kv16a = kv16[:, 0:HWTOT]
    kv16b = kv16[0:64, HWTOT:2 * HWTOT]
    k16 = big.tile([128, 2 * HWTOT], F16, tag="slotC", name="k16")
    k16a = k16[:, 0:HWTOT]          # k channels 0-127   (mc0)
    k16b = k16[0:64, HWTOT:]        # k channels 128-191 (mc1 lower)

    # small persistent
    qpart = sml.tile([128, NCH], F32)
    qpartb = sml.tile([64, NCH], F32)
    kpart = sml.tile([128, NCH], F32)
    kpartb = sml.tile([64, NCH], F32)
    spa = sml.tile([128, 1], F32)
    spb = sml.tile([64, 1], F32)
    invka = sml.tile([128, 1], F32)
    invkb = sml.tile([64, 1], F32)
    invk16a = sml.tile([128, 1], F16)
    invk16b = sml.tile([64, 1], F16)
    invkrow = sml.tile([1, C], F16)
    at16a = sml.tile([128, C], F16)
    at16b = sml.tile([64, C], F16)
    mt16a = sml.tile([128, C], F16)
    mt16b = sml.tile([64, C], F16)
    ikba = sml.tile([128, C], F32)
    ikbb = sml.tile([64, C], F32)

    # ---- load kv (cast f32 -> f16 in DMA) ----
    for ch in range(4):
        nc.gpsimd.dma_start(kv16a[:, sl(ch, 4096)], kv[0:128, sl(ch, 4096)])
    for ch in range(4):
        nc.gpsimd.dma_start(kv16b[:, sl(ch, 4096)], kv[128:C, sl(ch, 4096)])

    # Gram accumulators live across the whole conv/dw + gram phase
    psG = _stack.enter_context(tc.tile_pool(name="psG", bufs=1, space="PSUM"))
    G0 = psG.tile([128, C], F32, tag="G0", name="G0")
    G1 = psG.tile([64, C], F32, tag="G1", name="G1")

    gram_mm = [0]  # count of emitted Gram accumulation steps (of 2*H)

    # ================= conv1 + depthwise, mc blocks =================
    with tc.tile_pool(name="psC", bufs=3, space="PSUM") as psC, \
         tc.tile_pool(name="psD", bufs=3, space="PSUM") as psD, \
         tc.tile_pool(name="kvfp", bufs=1) as kvfp, \
         tc.tile_pool(name="qstage", bufs=2) as qst, \
         tc.tile_pool(name="vst", bufs=3) as vst:

        CHB = TCH * C  # 3072 f16 elems per chunk buffer

        def emit_q_chunk(c):
            qsa = qst.tile([128, CHW], F16, tag="qsa")
            nc.gpsimd.dma_start(qsa[:], q[0:128, sl(c, CHW)])
            qsb = qst.tile([64, CHW], F16, tag="qsb")
            nc.gpsimd.dma_start(qsb[:], q[128:C, sl(c, CHW)])
            qsq = qst.tile([128, CHW], F16, tag="sqt")
            nc.scalar.activation(qsq[:], qsa[:],
                                 mybir.ActivationFunctionType.Square,
                                 accum_out=qpart[:, c:c + 1])
            nc.scalar.activation(qsq[0:64, :], qsb[:],
                                 mybir.ActivationFunctionType.Square,
                                 accum_out=qpartb[:, c:c + 1])
            return qsa, qsb

        def emit_gram_chunk(c, qsa, qsb):
            b = c % 2
            qB = chunks[:, (2 * b) * CHB:(2 * b + 1) * CHB].rearrange(
                "p (t c) -> p t c", c=C)
            kB = chunks[:, (2 * b + 1) * CHB:(2 * b + 2) * CHB].rearrange(
                "p (t c) -> p t c", c=C)
            nc.sync.dma_start_transpose(qB[:, :, 0:128], qsa[:])
            nc.sync.dma_start_transpose(qB[:, :, 128:C], qsb[:])
            nc.sync.dma_start_transpose(kB[:, :, 0:128], k16a[:, sl(c, CHW)])
            nc.sync.dma_start_transpose(kB[:, :, 128:C], k16b[:, sl(c, CHW)])
            for t in range(TCH):
                s0 = gram_mm[0] == 0
                s1 = gram_mm[0] == H - 1
                nc.tensor.matmul(G0[:], qB[:, t, 0:128], kB[:, t, :],
                                 start=s0, stop=s1)
                nc.tensor.matmul(G1[:], qB[:, t, 128:C], kB[:, t, :],
                                 start=s0, stop=s1)
                gram_mm[0] += 1
        for mci, mc in enumerate(MC_ORDER):
            kvf = kvfp.tile([128, LR * LC], F16, tag="kvf", name="kvf")
            kvf3 = kvf[:].rearrange("p (r c) -> p r c", c=LC)
            nc.vector.memset(kvf3[:, 0, :], 0.0)
            nc.vector.memset(kvf3[:, LR - 1, :], 0.0)
            nc.vector.memset(kvf3[:, :, 0:1], 0.0)
            nc.vector.memset(kvf3[:, :, LC - 1:LC], 0.0)
            # conv1 for this mc block
            for j in range(NT):
                ps = psC.tile([128, PT], F32, tag="psC")
                nc.tensor.matmul(ps[:], w1ta[:, mc * 128:(mc + 1) * 128],
                                 kv16a[:, sl(j)], start=True, stop=False)
                nc.tensor.matmul(ps[:], w1tb[:, mc * 128:(mc + 1) * 128],
                                 kv16b[:, sl(j)], start=False, stop=True)
                dst = kvf3[:, 1 + j * RPT:1 + (j + 1) * RPT, 1:1 + W]
                nc.any.tensor_copy(dst, ps[:])
            # depthwise 3x3: 9 diag matmuls per pixel tile, PSUM accumulate
            for j in range(NT):
                r0 = j * RPT
                pd = psD.tile([128, PT], F32, tag="psD")
                for ti, (dr, dc) in enumerate(TAPS):
                    wi = (dr + 1) * 3 + (dc + 1)
                    rhs = kvf3[:, 1 + r0 + dr:1 + r0 + dr + RPT,
                               1 + dc:1 + dc + W]
                    nc.tensor.matmul(pd[:], w2sb[:, mc * 9 + wi, :], rhs,
                                     start=(ti == 0), stop=(ti == 8))
                # evacuate to destination by mc block
                if mc == 0:
                    nc.any.tensor_copy(k16a[:, sl(j)], pd[:])
                elif mc == 1:
                    nc.any.tensor_copy(k16b[:, sl(j)], pd[0:64, :])
                    vs = vst.tile([128, PT], F16, tag="vs")
                    nc.any.tensor_copy(vs[64:128, :], pd[64:128, :])
                    nc.sync.dma_start(vdram[0:64, sl(j)], vs[64:128, :])
                else:
                    vs = vst.tile([128, PT], F16, tag="vs")
                    nc.any.tensor_copy(vs[:], pd[:])
                    nc.sync.dma_start(vdram[64:C, sl(j)], vs[:])
            # after mc1 (first block): k16b is complete -> its squares
            if mci == 0:
                for c in range(NCH):
                    nc.scalar.activation(
                        qst.tile([64, CHW], F16, tag="sqt", name="ksq")[:],
                        k16b[:, sl(c, CHW)],
                        mybir.ActivationFunctionType.Square,
                        accum_out=kpartb[:, c:c + 1])
        # mc0 epilogue (inside pool scope): q pipeline + k16a squares + Gram.
        # Transposed q/k Gram chunks live in kv16's columns (kv16 is dead
        # after the last conv1 block); 2-deep double buffering via views.
        chunks = big.tile([128, 4 * CHB], F16, tag="slotA", name="chunks")
        for c in range(NCH):
            qsa, qsb = emit_q_chunk(c)
            nc.scalar.activation(
                qst.tile([128, CHW], F16, tag="sqt", name="ksqa")[:],
                k16a[:, sl(c, CHW)],
                mybir.ActivationFunctionType.Square,
                accum_out=kpart[:, c:c + 1])
            emit_gram_chunk(c, qsa, qsb)

    # ================= norms -> sp, invk =================
    nq2a = sml.tile([128, 1], F32)
    nq2b = sml.tile([64, 1], F32)
    nc.vector.reduce_sum(nq2a[:], qpart[:], axis=mybir.AxisListType.X)
    nc.vector.reduce_sum(nq2b[:], qpartb[:], axis=mybir.AxisListType.X)
    nc.vector.reduce_sum(invka[:], kpart[:], axis=mybir.AxisListType.X)
    nc.vector.reduce_sum(invkb[:], kpartb[:], axis=mybir.AxisListType.X)
    for nrm, scx, dst in ((nq2a, sca, spa), (nq2b, scb, spb)):
        nc.scalar.sqrt(nrm[:], nrm[:])
        nc.vector.tensor_scalar_max(nrm[:], nrm[:], EPS)
        nc.vector.reciprocal(nrm[:], nrm[:])
        nc.vector.tensor_tensor(out=dst[:], in0=nrm[:], in1=scx[:],
                                op=mybir.AluOpType.mult)
    for nrm in (invka, invkb):
        nc.scalar.sqrt(nrm[:], nrm[:])
        nc.vector.tensor_scalar_max(nrm[:], nrm[:], EPS)
        nc.vector.reciprocal(nrm[:], nrm[:])
    # broadcast 1/|k_j| along partitions: transpose to a row, outer-product
    nc.vector.tensor_copy(invk16a[:], invka[:])
    nc.vector.tensor_copy(invk16b[:], invkb[:])
    with tc.tile_pool(name="psB", bufs=1, space="PSUM") as psB:
        rowp = psB.tile([1, 128], F16, tag="rp", name="rowp")
        nc.tensor.transpose(rowp[:], invk16a[:], id16[:])
        nc.vector.tensor_copy(invkrow[0:1, 0:128], rowp[:])
        rowp2 = psB.tile([1, 128], F16, tag="rp2", name="rowp2")
        nc.tensor.transpose(rowp2[0:1, 0:64], invk16b[:],
                            id16[0:64, 0:64])
        nc.vector.tensor_copy(invkrow[0:1, 128:C], rowp2[0:1, 0:64])
        ikA = psB.tile([128, C], F32, tag="ikA", name="ikA")
        nc.tensor.matmul(ikA[:], ones1[0:1, 0:128], invkrow[:],
                         start=True, stop=True)
        ikB = psB.tile([64, C], F32, tag="ikB", name="ikB")
        nc.tensor.matmul(ikB[:], ones1[0:1, 0:64], invkrow[:],
                         start=True, stop=True)
        nc.vector.tensor_copy(ikba[:], ikA[:])
        nc.vector.tensor_copy(ikbb[:], ikB[:])

    # ================= logits, softmax, M^T = A^T Wp^T =================
    with tc.tile_pool(name="smax", bufs=1) as sm, \
         tc.tile_pool(name="psM", bufs=1, space="PSUM") as psM:
        for Gx, ikx, spx, mkx, atx, rows in (
                (G0, ikba, spa, maska, at16a, 128),
                (G1, ikbb, spb, maskb, at16b, 64)):
            lg = sm.tile([rows, C], F32, tag=f"lg{rows}", name=f"lg{rows}")
            nc.vector.tensor_tensor(out=lg[:], in0=Gx[:], in1=ikx[:],
                                    op=mybir.AluOpType.mult)
            lg2 = sm.tile([rows, C], F32, tag=f"lh{rows}", name=f"lh{rows}")
            nc.vector.scalar_tensor_tensor(
                out=lg2[:], in0=lg[:], scalar=spx[:], in1=mkx[:],
                op0=mybir.AluOpType.mult, op1=mybir.AluOpType.add)
            mx = sm.tile([rows, 1], F32, tag=f"mx{rows}", name=f"mx{rows}")
            nc.vector.reduce_max(mx[:], lg2[:], axis=mybir.AxisListType.X)
            nc.vector.tensor_scalar_mul(mx[:], mx[:], -1.0)
            ssum = sm.tile([rows, 1], F32, tag=f"ss{rows}", name=f"ss{rows}")
            nc.scalar.activation(lg2[:], lg2[:],
                                 mybir.ActivationFunctionType.Exp,
                                 bias=mx[:], accum_out=ssum[:])
            nc.vector.reciprocal(ssum[:], ssum[:])
            nc.vector.tensor_scalar_mul(atx[:], lg2[:], ssum[:])
        # MT = A^T @ Wp^T  ([d, o], d on partitions)
        mta = psM.tile([128, C], F32, tag="mta", name="mta")
        nc.tensor.matmul(mta[:], at16a[:, 0:128], wpta[:],
                         start=True, stop=False)
        nc.tensor.matmul(mta[:], at16b[:, 0:128], wptb[:],
                         start=False, stop=True)
        mtb = psM.tile([64, C], F32, tag="mtb", name="mtb")
        nc.tensor.matmul(mtb[:], at16a[:, 128:C], wpta[:],
                         start=True, stop=False)
        nc.tensor.matmul(mtb[:], at16b[:, 128:C], wptb[:],
                         start=False, stop=True)
        nc.scalar.copy(mt16a[:], mta[:])
        nc.scalar.copy(mt16b[:], mtb[:])

    # ================= out = M @ v =================
    with tc.tile_pool(name="vload", bufs=2) as vld, \
         tc.tile_pool(name="ost", bufs=2) as ost, \
         tc.tile_pool(name="psO", bufs=2, space="PSUM") as psO:
        for c in range(NCH):
            va = vld.tile([128, CHW], F16, tag="va")
            nc.sync.dma_start(va[:], vdram[0:128, sl(c, CHW)])
            vb = vld.tile([64, CHW], F16, tag="vb")
            nc.sync.dma_start(vb[:], vdram[128:C, sl(c, CHW)])
            oa = ost.tile([128, CHW], F32, tag="oa")
            ob = ost.tile([64, CHW], F32, tag="ob")
            for jj in range(CHW // PT):
                O0 = psO.tile([128, PT], F32, tag="O0")
                nc.tensor.matmul(O0[:], mt16a[:, 0:128], va[:, sl(jj)],
                                 start=True, stop=False)
                nc.tensor.matmul(O0[:], mt16b[:, 0:128], vb[:, sl(jj)],
                                 start=False, stop=True)
                O1 = psO.tile([64, PT], F32, tag="O1")
                nc.tensor.matmul(O1[:], mt16a[:, 128:C], va[:, sl(jj)],
                                 start=True, stop=False)
                nc.tensor.matmul(O1[:], mt16b[:, 128:C], vb[:, sl(jj)],
                                 start=False, stop=True)
                nc.any.tensor_copy(oa[:, sl(jj)], O0[:])
                nc.any.tensor_copy(ob[:, sl(jj)], O1[:])
            nc.sync.dma_start(out[0:128, sl(c, CHW)], oa[:])
            nc.sync.dma_start(out[128:C, sl(c, CHW)], ob[:])
    _stack.close()


def build_module():
    nc = bacc.Bacc("TRN2")
    io = {}
    io["kv"] = nc.dram_tensor("kv", [C, HWTOT], F32, kind="ExternalInput").ap()
    io["q"] = nc.dram_tensor("q", [C, HWTOT], F32, kind="ExternalInput").ap()
    io["w1t"] = nc.dram_tensor("w1t", [C, C2], F16, kind="ExternalInput").ap()
    io["w2d"] = nc.dram_tensor("w2d", [27, 128, 128], F16,
                               kind="ExternalInput").ap()
    io["wpt"] = nc.dram_tensor("wpt", [C, C], F16, kind="ExternalInput").ap()
    io["ident"] = nc.dram_tensor("ident", [128, 128], F16,
                                 kind="ExternalInput").ap()
    io["mask"] = nc.dram_tensor("mask", [C, C], F32, kind="ExternalInput").ap()
    io["scale192"] = nc.dram_tensor("scale192", [C, 1], F32,
                                    kind="ExternalInput").ap()
    io["out"] = nc.dram_tensor("out", [C, HWTOT], F32, kind="ExternalOutput").ap()
    io["vdram"] = nc.dram_tensor("vdram", [C, HWTOT], F16).ap()
    with tile.TileContext(nc) as tc:
        emit_kernel(tc, io)
    nc.compile()
    return nc


def prep_weights(qkv1_w, qkv2_w, proj_w, scale):
    w1 = np.asarray(qkv1_w).reshape(C2, C)
    w1t = np.ascontiguousarray(w1.T).astype(np.float16)
    w2 = np.asarray(qkv2_w).reshape(C2, 9)
    w2d = np.zeros((27, 128, 128), np.float16)
    for mc in range(3):
        for wi in range(9):
            np.fill_diagonal(w2d[mc * 9 + wi], w2[mc * 128:(mc + 1) * 128, wi])
    wpr = np.asarray(proj_w).reshape(C, C)
    wpt = np.ascontiguousarray(wpr.T).astype(np.float16)
    ident = np.eye(128, dtype=np.float16)
    mask = np.full((C, C), -1e30, np.float32)
    for h in range(HEADS):
        mask[h * CD:(h + 1) * CD, h * CD:(h + 1) * CD] = 0.0
    scale192 = np.repeat(np.asarray(scale).reshape(HEADS), CD).astype(
        np.float32).reshape(C, 1)
    return {"w1t": w1t, "w2d": w2d, "wpt": wpt, "ident": ident,
            "mask": mask, "scale192": scale192}


_CACHED = {}


def kernel(kv, q, qkv1_w, qkv2_w, proj_w, scale):
    kv = np.asarray(kv, np.float32)
    q = np.asarray(q, np.float32)
    b = kv.shape[0]
    assert b == 8 and kv.shape[1] == C
    wts = prep_weights(qkv1_w, qkv2_w, proj_w, scale)
    if "nc" not in _CACHED:
        ncm = build_module()
        ncm.m = get_hw_module(ncm.m)
        _CACHED["nc"] = ncm
    ncm = _CACHED["nc"]
    in_maps = []
    for i in range(b):
        m = {"kv": np.ascontiguousarray(kv[i].reshape(C, HWTOT)),
             "q": np.ascontiguousarray(q[i].reshape(C, HWTOT))}
        m.update(wts)
        in_maps.append(m)
    res = run_bass_kernel_spmd(ncm, in_maps, core_ids=list(range(8)))
    outv = np.stack([res.results[i]["out"].reshape(C, H, W) for i in range(b)])
    return outv.astype(np.float32)


# revision 17
# speedup vs baseline: 2.6415x; 1.1617x over previous
"""Trainium2 Bass kernel for nn_Cross_Attention (8-core data-parallel over batch).

Per batch item (one NeuronCore):
  kvf  = conv1x1(kv, qkv1_w)                    # [384, H, W]
  kvd  = depthwise3x3(kvf, qkv2_w, pad=1)       # [384, H, W]
  k, v = split(kvd)
  G    = q_raw @ k_raw^T  (full 192x192 Gram, contracted over pixels)
  attn = softmax(G * scale/|q_i| * 1/|k_j| + blockdiag_mask)
  out  = (Wp @ attn) @ v          # proj folded into attention matrix

Key structure:
 - depthwise as diagonal-weight matmuls with a row-padded kvf layout
   ([130 rows x 130 cols] per 128-ch block) so every tap's rhs is a
   [4,128]-stride-130 AP (256B runs -> full PE stream rate).
 - mc block order (1, 2, 0): k channels 128-191 finish early, k 0-127
   are produced last and Gram accumulation (on raw q/k) is interleaved
   chunk-by-chunk with the last depthwise block. Norms are folded into
   the logits afterwards (row scale = per-partition scalar; column
   scale 1/|k_j| via a K=1 outer-product matmul broadcast).
 - v spilled to DRAM as f16, read back during attn@v.
 - f32<->f16 casts ride inside SWDGE DMAs (gpsimd).
"""

import sys

sys.path.insert(0, "/opt/trn_rl_repo")

import numpy as np

import concourse.bass as bass
import concourse.tile as tile
from concourse import bacc, mybir
from concourse.bass_utils import run_bass_kernel_spmd
from concourse.bass_interp import get_hw_module

F32 = mybir.dt.float32
F16 = mybir.dt.float16

C = 192          # input channels
C2 = 384         # conv1 output channels
HEADS = 8
CD = C // HEADS
W = 128
H = 128
HWTOT = H * W    # 16384
PT = 512         # pixels per matmul tile
NT = HWTOT // PT  # 32
RPT = PT // W    # 4 rows per tile
LC = 130         # padded kvf cols (1 left pad, 1 right pad)
LR = 130         # padded kvf rows
EPS = 1e-12
MC_ORDER = (1, 2, 0)
NCH = 8          # norm/Gram chunks
CHW = HWTOT // NCH  # 2048 pixels per chunk
TCH = CHW // W   # 16 t-steps per chunk

TAPS = [(dr, dc) for dr in (-1, 0, 1) for dc in (-1, 0, 1)]


def sl(i, size=PT):
    return slice(i * size, (i + 1) * size)


def emit_kernel(tc, io):
    nc = tc.nc
    kv, q = io["kv"], io["q"]
    w1t, w2d, wpt, mask = io["w1t"], io["w2d"], io["wpt"], io["mask"]
    scale192, ident = io["scale192"], io["ident"]
    out, vdram = io["out"], io["vdram"]

    from contextlib import ExitStack
    _stack = ExitStack()
    wp = _stack.enter_context(tc.tile_pool(name="weights", bufs=1))
    big = _stack.enter_context(tc.tile_pool(name="big", bufs=1))
    sml = _stack.enter_context(tc.tile_pool(name="small", bufs=1))

    # ---- weights ----
    w1ta = wp.tile([128, C2], F16); nc.sync.dma_start(w1ta[:], w1t[0:128, :])
    w1tb = wp.tile([64, C2], F16); nc.sync.dma_start(w1tb[:], w1t[128:C, :])
    wpta = wp.tile([128, C], F16); nc.sync.dma_start(wpta[:], wpt[0:128, :])
    wptb = wp.tile([64, C], F16); nc.sync.dma_start(wptb[:], wpt[128:C, :])
    maska = wp.tile([128, C], F32); nc.sync.dma_start(maska[:], mask[0:128, :])
    maskb = wp.tile([64, C], F32); nc.sync.dma_start(maskb[:], mask[128:C, :])
    sca = wp.tile([128, 1], F32); nc.sync.dma_start(sca[:], scale192[0:128, :])
    scb = wp.tile([64, 1], F32); nc.sync.dma_start(scb[:], scale192[128:C, :])
    id16 = wp.tile([128, 128], F16); nc.sync.dma_start(id16[:], ident[:])
    w2sb = wp.tile([128, 27, 128], F16)
    nc.sync.dma_start(w2sb[:], w2d.rearrange("t p c -> p t c"))
    ones1 = wp.tile([1, 128], F16); nc.vector.memset(ones1[:], 1.0)

    # ---- big persistent tiles ----
    kv16 = big.tile([128, 2 * HWTOT], F16, tag="slotA", name="kv16")
    kv16a = kv16[:, 0:HWTOT]
    kv16b = kv16[0:64, HWTOT:2 * HWTOT]
    k16 = big.tile([128, 2 * HWTOT], F16, tag="slotC", name="k16")
    k16a = k16[:, 0:HWTOT]          # k channels 0-127   (mc0)
    k16b = k16[0:64, HWTOT:]        # k channels 128-191 (mc1 lower)

    # small persistent
    qpart = sml.tile([128, NCH], F32)
    qpartb = sml.tile([64, NCH], F32)
    kpart = sml.tile([128, NCH], F32)
    kpartb = sml.tile([64, NCH], F32)
    spa = sml.tile([128, 1], F32)
    spb = sml.tile([64, 1], F32)
    invka = sml.tile([128, 1], F32)
    invkb = sml.tile([64, 1], F32)
    invk16a = sml.tile([128, 1], F16)
    invk16b = sml.tile([64, 1], F16)
    invkrow = sml.tile([1, C], F16)
    at16a = sml.tile([128, C], F16)
    at16b = sml.tile([64, C], F16)
    mt16a = sml.tile([128, C], F16)
    mt16b = sml.tile([64, C], F16)
    ikba = sml.tile([128, C], F32)
    ikbb = sml.tile([64, C], F32)

    # ---- load kv (cast f32 -> f16 in DMA), a/b interleaved ----
    for ch in range(8):
        nc.gpsimd.dma_start(kv16a[:, sl(ch, 2048)], kv[0:128, sl(ch, 2048)])
        nc.gpsimd.dma_start(kv16b[:, sl(ch, 2048)], kv[128:C, sl(ch, 2048)])

    # Gram accumulators live across the whole conv/dw + gram phase
    psG = _stack.enter_context(tc.tile_pool(name="psG", bufs=1, space="PSUM"))
    G0 = psG.tile([128, C], F32, tag="G0", name="G0")
    G1 = psG.tile([64, C], F32, tag="G1", name="G1")

    gram_mm = [0]  # count of emitted Gram accumulation steps (of 2*H)

    # ================= conv1 + depthwise, mc blocks =================
    with tc.tile_pool(name="psC", bufs=3, space="PSUM") as psC, \
         tc.tile_pool(name="psD", bufs=3, space="PSUM") as psD, \
         tc.tile_pool(name="kvfp", bufs=1) as kvfp, \
         tc.tile_pool(name="qstage", bufs=2) as qst, \
         tc.tile_pool(name="vst", bufs=3) as vst:

        CHB = TCH * C  # 3072 f16 elems per chunk buffer

        def emit_q_chunk(c):
            qsa = qst.tile([128, CHW], F16, tag="qsa")
            nc.gpsimd.dma_start(qsa[:], q[0:128, sl(c, CHW)])
            qsb = qst.tile([64, CHW], F16, tag="qsb")
            nc.gpsimd.dma_start(qsb[:], q[128:C, sl(c, CHW)])
            qsq = qst.tile([128, CHW], F16, tag="sqt")
            nc.scalar.activation(qsq[:], qsa[:],
                                 mybir.ActivationFunctionType.Square,
                                 accum_out=qpart[:, c:c + 1])
            nc.scalar.activation(qsq[0:64, :], qsb[:],
                                 mybir.ActivationFunctionType.Square,
                                 accum_out=qpartb[:, c:c + 1])
            return qsa, qsb

        def chunk_views(c):
            b = c % 2
            qB = chunks[:, (2 * b) * CHB:(2 * b + 1) * CHB].rearrange(
                "p (t c) -> p t c", c=C)
            kB = chunks[:, (2 * b + 1) * CHB:(2 * b + 2) * CHB].rearrange(
                "p (t c) -> p t c", c=C)
            return qB, kB

        def emit_transposes(c, qsa, qsb):
            # q on the Scalar HWDGE queue, k on Sync: the two pairs prefetch
            # in parallel while the PE runs the next depthwise chunk.
            qB, kB = chunk_views(c)
            nc.scalar.dma_start_transpose(qB[:, :, 0:128], qsa[:])
            nc.scalar.dma_start_transpose(qB[:, :, 128:C], qsb[:])
            nc.sync.dma_start_transpose(kB[:, :, 0:128], k16a[:, sl(c, CHW)])
            nc.sync.dma_start_transpose(kB[:, :, 128:C], k16b[:, sl(c, CHW)])

        def emit_gram_mms(c):
            qB, kB = chunk_views(c)
            for t in range(TCH):
                s0 = gram_mm[0] == 0
                s1 = gram_mm[0] == H - 1
                nc.tensor.matmul(G0[:], qB[:, t, 0:128], kB[:, t, :],
                                 start=s0, stop=s1)
                nc.tensor.matmul(G1[:], qB[:, t, 128:C], kB[:, t, :],
                                 start=s0, stop=s1)
                gram_mm[0] += 1
        for mci, mc in enumerate(MC_ORDER):
            kvf = kvfp.tile([128, LR * LC], F16, tag="kvf", name="kvf")
            kvf3 = kvf[:].rearrange("p (r c) -> p r c", c=LC)
            nc.vector.memset(kvf3[:, 0, :], 0.0)
            nc.vector.memset(kvf3[:, LR - 1, :], 0.0)
            nc.vector.memset(kvf3[:, :, 0:1], 0.0)
            nc.vector.memset(kvf3[:, :, LC - 1:LC], 0.0)
            # conv1 for this mc block
            for j in range(NT):
                ps = psC.tile([128, PT], F32, tag="psC")
                nc.tensor.matmul(ps[:], w1ta[:, mc * 128:(mc + 1) * 128],
                                 kv16a[:, sl(j)], start=True, stop=False)
                nc.tensor.matmul(ps[:], w1tb[:, mc * 128:(mc + 1) * 128],
                                 kv16b[:, sl(j)], start=False, stop=True)
                dst = kvf3[:, 1 + j * RPT:1 + (j + 1) * RPT, 1:1 + W]
                nc.any.tensor_copy(dst, ps[:])
            # depthwise 3x3: 9 diag matmuls per pixel tile, PSUM accumulate
            def dw_tile(j):
                r0 = j * RPT
                pd = psD.tile([128, PT], F32, tag="psD", name="pd")
                for ti, (dr, dc) in enumerate(TAPS):
                    wi = (dr + 1) * 3 + (dc + 1)
                    rhs = kvf3[:, 1 + r0 + dr:1 + r0 + dr + RPT,
                               1 + dc:1 + dc + W]
                    nc.tensor.matmul(pd[:], w2sb[:, mc * 9 + wi, :], rhs,
                                     start=(ti == 0), stop=(ti == 8))
                # evacuate to destination by mc block
                if mc == 0:
                    nc.vector.tensor_copy(k16a[:, sl(j)], pd[:])
                elif mc == 1:
                    nc.any.tensor_copy(k16b[:, sl(j)], pd[0:64, :])
                    vs = vst.tile([128, PT], F16, tag="vs", name="vs")
                    nc.any.tensor_copy(vs[64:128, :], pd[64:128, :])
                    nc.sync.dma_start(vdram[0:64, sl(j)], vs[64:128, :])
                else:
                    vs = vst.tile([128, PT], F16, tag="vs", name="vs")
                    nc.any.tensor_copy(vs[:], pd[:])
                    nc.sync.dma_start(vdram[64:C, sl(j)], vs[:])

            if mc != 0:
                for j in range(NT):
                    dw_tile(j)
            else:
                # interleave: dw chunk c -> transposes c -> gram MMs c-1
                for c in range(NCH):
                    for j in range(4 * c, 4 * c + 4):
                        dw_tile(j)
                    qsa, qsb = qstaged[c]
                    emit_transposes(c, qsa, qsb)
                    nc.scalar.activation(
                        qst.tile([128, CHW], F16, tag="sqt", name="ksqa")[:],
                        k16a[:, sl(c, CHW)],
                        mybir.ActivationFunctionType.Square,
                        accum_out=kpart[:, c:c + 1])
                    if c >= 1:
                        emit_gram_mms(c - 1)
                emit_gram_mms(NCH - 1)
            if mci == 0:
                # after mc1 (first block): k16b complete -> its squares
                for c in range(NCH):
                    nc.scalar.activation(
                        qst.tile([64, CHW], F16, tag="sqt", name="ksq")[:],
                        k16b[:, sl(c, CHW)],
                        mybir.ActivationFunctionType.Square,
                        accum_out=kpartb[:, c:c + 1])
            elif mci == 1:
                # during/after mc2: q loads + squares (buffer rotation paces
                # the loads against mc0's transposes)
                chunks = big.tile([128, 4 * CHB], F16, tag="slotA",
                                  name="chunks")
                qstaged = [emit_q_chunk(c) for c in range(NCH)]

    # ---- prefetch all of v into k16's columns (k16 dead after the
    # last kB transpose); overlaps the gram/logits tail ----
    vfull = big.tile([128, 2 * HWTOT], F16, tag="slotC", name="vfull")
    va_all = vfull[:, 0:HWTOT]
    vb_all = vfull[0:64, HWTOT:]
    for ch in range(4):
        nc.scalar.dma_start(va_all[:, sl(ch, 4096)], vdram[0:128, sl(ch, 4096)])
    for ch in range(4):
        nc.scalar.dma_start(vb_all[:, sl(ch, 4096)], vdram[128:C, sl(ch, 4096)])

    # ================= norms -> sp, invk =================
    nq2a = sml.tile([128, 1], F32)
    nq2b = sml.tile([64, 1], F32)
    nc.vector.reduce_sum(nq2a[:], qpart[:], axis=mybir.AxisListType.X)
    nc.vector.reduce_sum(nq2b[:], qpartb[:], axis=mybir.AxisListType.X)
    nc.vector.reduce_sum(invka[:], kpart[:], axis=mybir.AxisListType.X)
    nc.vector.reduce_sum(invkb[:], kpartb[:], axis=mybir.AxisListType.X)
    for nrm, scx, dst in ((nq2a, sca, spa), (nq2b, scb, spb)):
        nc.scalar.sqrt(nrm[:], nrm[:])
        nc.vector.tensor_scalar_max(nrm[:], nrm[:], EPS)
        nc.vector.reciprocal(nrm[:], nrm[:])
        nc.vector.tensor_tensor(out=dst[:], in0=nrm[:], in1=scx[:],
                                op=mybir.AluOpType.mult)
    for nrm in (invka, invkb):
        nc.scalar.sqrt(nrm[:], nrm[:])
        nc.vector.tensor_scalar_max(nrm[:], nrm[:], EPS)
        nc.vector.reciprocal(nrm[:], nrm[:])
    # broadcast 1/|k_j| along partitions: transpose to a row, outer-product
    nc.vector.tensor_copy(invk16a[:], invka[:])
    nc.vector.tensor_copy(invk16b[:], invkb[:])
    with tc.tile_pool(name="psB", bufs=1, space="PSUM") as psB:
        rowp = psB.tile([1, 128], F16, tag="rp", name="rowp")
        nc.tensor.transpose(rowp[:], invk16a[:], id16[:])
        nc.vector.tensor_copy(invkrow[0:1, 0:128], rowp[:])
        rowp2 = psB.tile([1, 128], F16, tag="rp2", name="rowp2")
        nc.tensor.transpose(rowp2[0:1, 0:64], invk16b[:],
                            id16[0:64, 0:64])
        nc.vector.tensor_copy(invkrow[0:1, 128:C], rowp2[0:1, 0:64])
        ikA = psB.tile([128, C], F32, tag="ikA", name="ikA")
        nc.tensor.matmul(ikA[:], ones1[0:1, 0:128], invkrow[:],
                         start=True, stop=True)
        ikB = psB.tile([64, C], F32, tag="ikB", name="ikB")
        nc.tensor.matmul(ikB[:], ones1[0:1, 0:64], invkrow[:],
                         start=True, stop=True)
        nc.vector.tensor_copy(ikba[:], ikA[:])
        nc.vector.tensor_copy(ikbb[:], ikB[:])

    # ================= logits, softmax, M^T = A^T Wp^T =================
    with tc.tile_pool(name="smax", bufs=1) as sm, \
         tc.tile_pool(name="psM", bufs=1, space="PSUM") as psM:
        for Gx, ikx, spx, mkx, atx, rows in (
                (G0, ikba, spa, maska, at16a, 128),
                (G1, ikbb, spb, maskb, at16b, 64)):
            lg = sm.tile([rows, C], F32, tag=f"lg{rows}", name=f"lg{rows}")
            nc.vector.tensor_tensor(out=lg[:], in0=Gx[:], in1=ikx[:],
                                    op=mybir.AluOpType.mult)
            lg2 = sm.tile([rows, C], F32, tag=f"lh{rows}", name=f"lh{rows}")
            nc.vector.scalar_tensor_tensor(
                out=lg2[:], in0=lg[:], scalar=spx[:], in1=mkx[:],
                op0=mybir.AluOpType.mult, op1=mybir.AluOpType.add)
            mx = sm.tile([rows, 1], F32, tag=f"mx{rows}", name=f"mx{rows}")
            nc.vector.reduce_max(mx[:], lg2[:], axis=mybir.AxisListType.X)
            nc.vector.tensor_scalar_mul(mx[:], mx[:], -1.0)
            ssum = sm.tile([rows, 1], F32, tag=f"ss{rows}", name=f"ss{rows}")
            nc.scalar.activation(lg2[:], lg2[:],
                                 mybir.ActivationFunctionType.Exp,
                                 bias=mx[:], accum_out=ssum[:])
            nc.vector.reciprocal(ssum[:], ssum[:])
            nc.vector.tensor_scalar_mul(atx[:], lg2[:], ssum[:])
        # MT = A^T @ Wp^T  ([d, o], d on partitions)
        mta = psM.tile([128, C], F32, tag="mta", name="mta")
        nc.tensor.matmul(mta[:], at16a[:, 0:128], wpta[:],
                         start=True, stop=False)
        nc.tensor.matmul(mta[:], at16b[:, 0:128], wptb[:],
                         start=False, stop=True)
        mtb = psM.tile([64, C], F32, tag="mtb", name="mtb")
        nc.tensor.matmul(mtb[:], at16a[:, 128:C], wpta[:],
                         start=True, stop=False)
        nc.tensor.matmul(mtb[:], at16b[:, 128:C], wptb[:],
                         start=False, stop=True)
        nc.scalar.copy(mt16a[:], mta[:])
        nc.scalar.copy(mt16b[:], mtb[:])

    # ================= out = M @ v =================
    with tc.tile_pool(name="ost", bufs=2) as ost, \
         tc.tile_pool(name="psO", bufs=2, space="PSUM") as psO:
        for c in range(NCH):
            oa = ost.tile([128, CHW], F32, tag="oa")
            ob = ost.tile([64, CHW], F32, tag="ob")
            for jj in range(CHW // PT):
                j = c * (CHW // PT) + jj
                O0 = psO.tile([128, PT], F32, tag="O0")
                nc.tensor.matmul(O0[:], mt16a[:, 0:128], va_all[:, sl(j)],
                                 start=True, stop=False)
                nc.tensor.matmul(O0[:], mt16b[:, 0:128], vb_all[:, sl(j)],
                                 start=False, stop=True)
                O1 = psO.tile([64, PT], F32, tag="O1")
                nc.tensor.matmul(O1[:], mt16a[:, 128:C], va_all[:, sl(j)],
                                 start=True, stop=False)
                nc.tensor.matmul(O1[:], mt16b[:, 128:C], vb_all[:, sl(j)],
                                 start=False, stop=True)
                nc.any.tensor_copy(oa[:, sl(jj)], O0[:])
                nc.any.tensor_copy(ob[:, sl(jj)], O1[:])
            nc.sync.dma_start(out[0:128, sl(c, CHW)], oa[:])
            nc.sync.dma_start(out[128:C, sl(c, CHW)], ob[:])
    _stack.close()


def build_module():
    nc = bacc.Bacc("TRN2")
    io = {}
    io["kv"] = nc.dram_tensor("kv", [C, HWTOT], F32, kind="ExternalInput").ap()
    io["q"] = nc.dram_tensor("q", [C, HWTOT], F32, kind="ExternalInput").ap()
    io["w1t"] = nc.dram_tensor("w1t", [C, C2], F16, kind="ExternalInput").ap()
    io["w2d"] = nc.dram_tensor("w2d", [27, 128, 128], F16,
                               kind="ExternalInput").ap()
    io["wpt"] = nc.dram_tensor("wpt", [C, C], F16, kind="ExternalInput").ap()
    io["ident"] = nc.dram_tensor("ident", [128, 128], F16,
                                 kind="ExternalInput").ap()
    io["mask"] = nc.dram_tensor("mask", [C, C], F32, kind="ExternalInput").ap()
    io["scale192"] = nc.dram_tensor("scale192", [C, 1], F32,
                                    kind="ExternalInput").ap()
    io["out"] = nc.dram_tensor("out", [C, HWTOT], F32, kind="ExternalOutput").ap()
    io["vdram"] = nc.dram_tensor("vdram", [C, HWTOT], F16).ap()
    with tile.TileContext(nc) as tc:
        emit_kernel(tc, io)
    nc.compile()
    return nc


def prep_weights(qkv1_w, qkv2_w, proj_w, scale):
    w1 = np.asarray(qkv1_w).reshape(C2, C)
    w1t = np.ascontiguousarray(w1.T).astype(np.float16)
    w2 = np.asarray(qkv2_w).reshape(C2, 9)
    w2d = np.zeros((27, 128, 128), np.float16)
    for mc in range(3):
        for wi in range(9):
            np.fill_diagonal(w2d[mc * 9 + wi], w2[mc * 128:(mc + 1) * 128, wi])
    wpr = np.asarray(proj_w).reshape(C, C)
    wpt = np.ascontiguousarray(wpr.T).astype(np.float16)
    ident = np.eye(128, dtype=np.float16)
    mask = np.full((C, C), -1e30, np.float32)
    for h in range(HEADS):
        mask[h * CD:(h + 1) * CD, h * CD:(h + 1) * CD] = 0.0
    scale192 = np.repeat(np.asarray(scale).reshape(HEADS), CD).astype(
        np.float32).reshape(C, 1)
    return {"w1t": w1t, "w2d": w2d, "wpt": wpt, "ident": ident,
            "mask": mask, "scale192": scale192}


_CACHED = {}


def kernel(kv, q, qkv1_w, qkv2_w, proj_w, scale):
    kv = np.asarray(kv, np.float32)
    q = np.asarray(q, np.float32)
    b = kv.shape[0]
    assert b == 8 and kv.shape[1] == C
    wts = prep_weights(qkv1_w, qkv2_w, proj_w, scale)
    if "nc" not in _CACHED:
        ncm = build_module()
        ncm.m = get_hw_module(ncm.m)
        _CACHED["nc"] = ncm
    ncm = _CACHED["nc"]
    in_maps = []
    for i in range(b):
        m = {"kv": np.ascontiguousarray(kv[i].reshape(C, HWTOT)),
             "q": np.ascontiguousarray(q[i].reshape(C, HWTOT))}
        m.update(wts)
        in_maps.append(m)
    res = run_bass_kernel_spmd(ncm, in_maps, core_ids=list(range(8)))
    outv = np.stack([res.results[i]["out"].reshape(C, H, W) for i in range(b)])
    return outv.astype(np.float32)
